# revision 1
# baseline (speedup 1.0000x reference)
"""EntailmentConeLoss on 8 Trainium2 NeuronCores.

Data-parallel over pairs (8192 pos + 32768 neg per core), prototype table
replicated in bf16 (tolerance 2e-2; dots of 256-dim bf16 rows carry ~0.3%
error and the loss averages 327680 energies, so bf16 is safe).

Per core:
- Rows are fetched with gpsimd dma_gather(transpose=True) on a 4-row-strided
  bf16 table view (int16 quotient indices, residue per bucket). Each gathered
  row is 512B and lands TRANSPOSED: tile [128, 2, n] holds element 128*t+p of
  pair-column j at [p, t, j]. Pairs are bucket-sorted by (a%4, c%4) on the
  host; one gather call per bucket per role (64 calls/core). Buckets use
  per-bucket capacities (max over cores, rounded to 128). Gather tiles are
  triple-buffered so bucket k+2's descriptor generation does not wait on
  bucket k's elementwise reads (WAR) -- this keeps the DMA engines streaming.
- Elementwise a*c / a^2 / c^2 in bf16 (DVE tensor_tensor at 2x, ACT Square).
- Reduction over D uses the TensorEngine: per 128-pair block, the operand
  tile is the STATIONARY matmul input [128 d-half, 128 pairs] and a ones
  column is moving, so psum[:, blk] holds per-pair dots, accumulated over the
  two d-halves. Pair j lands at psum partition j%128, col j//128.
- f32 epilogue (octant-reduced arccos, arcsin series) runs in three chunks
  (pos, neg buckets 0-7, neg buckets 8-15), each as soon as its psum group
  completes, overlapping the remaining stream. partials [128, 4]: col0 = pos
  energy sums, col1+col2 = neg hinge sums; host combines.
"""
import os
os.environ.setdefault("NEURON_RT_RESET_CORES", "1")

import numpy as np

C, D = 100000, 256
P_TOT, K = 65536, 4
NCORES = 8
PPC = P_TOT // NCORES          # pos pairs per core
NPC = PPC * K                  # neg pairs per core
NBUCK = 16
EPS = np.float32(1e-6)
BETA = np.float32(0.1)
MARGIN = np.float32(0.1)
QUEUES = int(os.environ.get("KQ", "4"))
TBL_FP8 = os.environ.get("KFP8", "0") == "1"
SQC_MODE = int(os.environ.get("KSQC", "3"))   # sqc on ACT every Nth (0=always ACT)
SINGLE_PACKET = os.environ.get("KSP", "0") == "1"
GAT_BUFS = int(os.environ.get("KGB", "4"))
EW_BUFS = int(os.environ.get("KEB", "2"))

_CACHE = {}


def _build_program(caps_p, caps_n, loop_iters=1, stage=5):
    import concourse.bass as bass
    import concourse.bacc as bacc
    import concourse.mybir as mybir
    import concourse.tile as tile

    f32 = mybir.dt.float32
    bf16 = mybir.dt.bfloat16
    gdt = mybir.dt.float8e4 if TBL_FP8 else bf16
    i16 = mybir.dt.int16
    Alu = mybir.AluOpType
    Act = mybir.ActivationFunctionType

    caps_p = list(caps_p)
    caps_n = list(caps_n)
    cum_p = np.concatenate([[0], np.cumsum(caps_p)]).astype(int)
    cum_n = np.concatenate([[0], np.cumsum(caps_n)]).astype(int)
    NPOS = int(cum_p[-1])
    NNEG = int(cum_n[-1])
    NPB = NPOS // 128              # pos 128-pair blocks
    NNB = NNEG // 128
    NB = NPB + NNB

    nc = bacc.Bacc("TRN2", target_bir_lowering=False, num_devices=NCORES,
                   num_swdge_queues=4)
    tbl = nc.dram_tensor("tblbf", [C, D], gdt, kind="ExternalInput")
    posa_i = nc.dram_tensor("posa_i", [128, NPOS // 16], i16, kind="ExternalInput")
    posb_i = nc.dram_tensor("posb_i", [128, NPOS // 16], i16, kind="ExternalInput")
    nega_i = nc.dram_tensor("nega_i", [128, NNEG // 16], i16, kind="ExternalInput")
    negc_i = nc.dram_tensor("negc_i", [128, NNEG // 16], i16, kind="ExternalInput")
    maskp = nc.dram_tensor("maskp", [128, NPB], f32, kind="ExternalInput")
    maskn = nc.dram_tensor("maskn", [128, NNB], f32, kind="ExternalInput")
    partials = nc.dram_tensor("partials", [128, 4], f32, kind="ExternalOutput")

    HALF_PI = float(np.float32(np.pi / 2))

    with tile.TileContext(nc) as tc:
        with tc.tile_pool(name="io", bufs=1) as io, \
             tc.tile_pool(name="gat", bufs=GAT_BUFS) as gat, \
             tc.tile_pool(name="ew", bufs=EW_BUFS) as ew, \
             tc.tile_pool(name="ps", bufs=1, space="PSUM") as ps, \
             tc.tile_pool(name="tmp", bufs=1) as tmp:

            posa_t = io.tile([128, NPOS // 16], i16)
            posb_t = io.tile([128, NPOS // 16], i16)
            nega_t = io.tile([128, NNEG // 16], i16)
            negc_t = io.tile([128, NNEG // 16], i16)
            maskp_t = io.tile([128, NPB], f32)
            maskn_t = io.tile([128, NNB], f32)
            nc.sync.dma_start(out=posa_t[:], in_=posa_i[:])
            nc.sync.dma_start(out=posb_t[:], in_=posb_i[:])
            nc.sync.dma_start(out=nega_t[:], in_=nega_i[:])
            nc.sync.dma_start(out=negc_t[:], in_=negc_i[:])
            nc.sync.dma_start(out=maskp_t[:], in_=maskp[:])
            nc.sync.dma_start(out=maskn_t[:], in_=maskn[:])

            ones_t = io.tile([128, 1], bf16)
            nc.vector.memset(ones_t[:], 1.0)

            NH0 = int(cum_n[NBUCK // 2]) // 128
            NH1 = NNB - NH0
            sb_p = io.tile([128, 3 * NPB], f32)
            sb_n0 = io.tile([128, 3 * NH0], f32)
            sb_n1 = io.tile([128, 3 * NH1], f32)
            out_t = io.tile([128, 4], f32)
            nc.vector.memset(out_t[:], 0.0)

            tview = tbl[:].rearrange("(q r) d -> q r d", r=4)

            qrr = [0]
            sqc_rr = [0]

            def loop_body(_i=None):
                # psum group tiles: [pp cols | cc cols | pc cols], one bank each
                def psgroup(tag, ncols):
                    return ps.tile([128, 3 * ncols], f32, tag=tag, name=tag)

                def stream(a_idx_t, c_idx_t, caps, cum, grp, blkbase, tagp,
                           buckets):
                    gncol = grp.shape[1] // 3
                    for xy in buckets:
                        cap = int(caps[xy])
                        off16 = int(cum[xy]) // 16
                        ra, rc = xy // 4, xy % 4
                        A = gat.tile([128, 2, cap], gdt, tag=tagp + "ga",
                                     name=tagp + "ga")
                        Cc = gat.tile([128, 2, cap], gdt, tag=tagp + "gc",
                                      name=tagp + "gc")
                        nc.gpsimd.dma_gather(
                            A[:], tview[:, ra, :],
                            a_idx_t[:, off16:off16 + cap // 16],
                            cap, cap, D, elem_step=4 * D, transpose=True,
                            single_packet=SINGLE_PACKET, queue_num=qrr[0] % QUEUES)
                        nc.gpsimd.dma_gather(
                            Cc[:], tview[:, rc, :],
                            c_idx_t[:, off16:off16 + cap // 16],
                            cap, cap, D, elem_step=4 * D, transpose=True,
                            single_packet=SINGLE_PACKET, queue_num=(qrr[0] + 1) % QUEUES)
                        qrr[0] += 2
                        if stage < 1:
                            continue
                        Af = A[:].rearrange("p a b -> p (a b)")
                        Cf = Cc[:].rearrange("p a b -> p (a b)")
                        prod = ew.tile([128, 2, cap], bf16, tag=tagp + "pr",
                                       name=tagp + "pr")
                        sqa = ew.tile([128, 2, cap], bf16, tag=tagp + "sa",
                                      name=tagp + "sa")
                        sqc = ew.tile([128, 2, cap], bf16, tag=tagp + "sc",
                                      name=tagp + "sc")
                        nc.vector.tensor_tensor(
                            out=prod[:].rearrange("p a b -> p (a b)"),
                            in0=Af, in1=Cf, op=Alu.mult)
                        nc.scalar.activation(
                            out=sqa[:].rearrange("p a b -> p (a b)"),
                            in_=Af, func=Act.Square)
                        # alternate engine for c^2 to balance DVE vs ACT
                        sqc_rr[0] += 1
                        if SQC_MODE == 0 or sqc_rr[0] % SQC_MODE == 0:
                            nc.scalar.activation(
                                out=sqc[:].rearrange("p a b -> p (a b)"),
                                in_=Cf, func=Act.Square)
                        else:
                            nc.vector.tensor_tensor(
                                out=sqc[:].rearrange("p a b -> p (a b)"),
                                in0=Cf, in1=Cf, op=Alu.mult)
                        if stage < 2:
                            continue
                        nblk = cap // 128
                        for t in range(nblk):
                            col = int(cum[xy]) // 128 + t - blkbase
                            for qi, tl in ((0, sqa), (1, sqc), (2, prod)):
                                pcol = qi * gncol + col
                                nc.tensor.matmul(
                                    grp[:, pcol:pcol + 1],
                                    tl[:, 0, t * 128:(t + 1) * 128],
                                    ones_t[:, 0:1],
                                    start=True, stop=False)
                                nc.tensor.matmul(
                                    grp[:, pcol:pcol + 1],
                                    tl[:, 1, t * 128:(t + 1) * 128],
                                    ones_t[:, 0:1],
                                    start=False, stop=True)

                # ---------------- epilogue (wide f32 ops) ----------------
                # stage 3: through cos; stage 4: + arccos; stage >=5: full
                def group_epilogue(grp, sbt, mask_ap, ncol, is_neg, out_col):
                    if stage < 2:
                        return
                    nc.vector.tensor_copy(sbt[:], grp[:])
                    if stage >= 3:
                        epilogue(sbt[:, 0:ncol], sbt[:, ncol:2 * ncol],
                                 sbt[:, 2 * ncol:3 * ncol], mask_ap, ncol,
                                 is_neg, out_col)

                def epilogue(pp_b, cc_b, pc_b, mask_t, ncol, is_neg, out_col):
                    T = lambda nm: tmp.tile([128, ncol], f32, tag="ep" + nm,
                                            name="ep" + nm)
                    ppcc = T("ppcc")
                    nc.vector.tensor_tensor(out=ppcc[:], in0=cc_b, in1=pp_b, op=Alu.add)
                    t2 = T("t2")
                    nc.vector.tensor_scalar(out=t2[:], in0=pc_b, scalar1=-2.0,
                                            scalar2=None, op0=Alu.mult)
                    dd = T("dd")
                    nc.vector.tensor_tensor(out=dd[:], in0=ppcc[:], in1=t2[:], op=Alu.add)
                    # near-duplicate guard: dd is rounding junk when c≈p (the
                    # three dots come from different engine paths); force
                    # cos=0 (ang=pi/2) like the reference's eps-denominator.
                    dupf = T("dupf")
                    nc.vector.tensor_scalar(out=dupf[:], in0=ppcc[:], scalar1=2e-3,
                                            scalar2=None, op0=Alu.mult)
                    nc.vector.tensor_tensor(out=dupf[:], in0=dd[:], in1=dupf[:], op=Alu.is_lt)
                    nc.vector.tensor_scalar(out=dupf[:], in0=dupf[:], scalar1=-1.0,
                                            scalar2=1.0, op0=Alu.mult, op1=Alu.add)
                    nc.vector.tensor_scalar(out=dd[:], in0=dd[:], scalar1=0.0,
                                            scalar2=None, op0=Alu.max)
                    g = T("g")
                    nc.vector.tensor_tensor(out=g[:], in0=pp_b, in1=dd[:], op=Alu.mult)
                    # s = sqrt(g) + one Newton step; +1e-30 keeps g=0 finite
                    nc.vector.tensor_scalar(out=g[:], in0=g[:], scalar1=1e-30,
                                            scalar2=None, op0=Alu.add)
                    s0 = T("s0")
                    nc.scalar.activation(out=s0[:], in_=g[:], func=Act.Sqrt)
                    r = T("r")
                    nc.vector.reciprocal(r[:], s0[:])
                    s1 = T("s1")
                    nc.vector.tensor_tensor(out=s1[:], in0=g[:], in1=r[:], op=Alu.mult)
                    nc.vector.tensor_tensor(out=s1[:], in0=s1[:], in1=s0[:], op=Alu.add)
                    den = T("den")
                    nc.vector.tensor_scalar(out=den[:], in0=s1[:], scalar1=float(EPS),
                                            scalar2=None, op0=Alu.add)
                    rden = T("rden")
                    nc.vector.reciprocal(rden[:], den[:])
                    num = T("num")
                    nc.vector.tensor_tensor(out=num[:], in0=pc_b, in1=pp_b, op=Alu.subtract)
                    cos = T("cos")
                    nc.vector.tensor_tensor(out=cos[:], in0=num[:], in1=rden[:], op=Alu.mult)
                    nc.vector.tensor_scalar(out=cos[:], in0=cos[:], scalar1=2.0,
                                            scalar2=float(-(1.0 - 1e-6)), op0=Alu.mult,
                                            op1=Alu.max)
                    nc.vector.tensor_scalar(out=cos[:], in0=cos[:], scalar1=float(1.0 - 1e-6),
                                            scalar2=None, op0=Alu.min)
                    nc.vector.tensor_tensor(out=cos[:], in0=cos[:], in1=dupf[:], op=Alu.mult)
                    if stage == 3:
                        nc.vector.tensor_tensor(out=cos[:], in0=cos[:], in1=mask_t, op=Alu.mult)
                        nc.vector.tensor_reduce(
                            out=out_t[:, out_col:out_col + 1], in_=cos[:],
                            axis=mybir.AxisListType.X, op=Alu.add)
                        return
                    # ang = arccos(cos) via octant-reduced arctan
                    q = T("q")
                    nc.vector.tensor_tensor(out=q[:], in0=cos[:], in1=cos[:], op=Alu.mult)
                    nc.vector.tensor_scalar(out=q[:], in0=q[:], scalar1=-1.0,
                                            scalar2=1.0, op0=Alu.mult, op1=Alu.add)
                    q0 = T("q0")
                    nc.scalar.activation(out=q0[:], in_=q[:], func=Act.Sqrt)
                    rq = T("rq")
                    nc.vector.reciprocal(rq[:], q0[:])
                    sq = T("sq")
                    nc.vector.tensor_tensor(out=sq[:], in0=q[:], in1=rq[:], op=Alu.mult)
                    nc.vector.tensor_tensor(out=sq[:], in0=sq[:], in1=q0[:], op=Alu.add)
                    nc.vector.tensor_scalar(out=sq[:], in0=sq[:], scalar1=0.5,
                                            scalar2=None, op0=Alu.mult)
                    abst = T("abst")
                    nc.vector.tensor_scalar(out=abst[:], in0=cos[:], scalar1=-1.0,
                                            scalar2=None, op0=Alu.mult)
                    nc.vector.tensor_tensor(out=abst[:], in0=abst[:], in1=cos[:], op=Alu.max)
                    u = T("u")
                    nc.vector.tensor_tensor(out=u[:], in0=abst[:], in1=sq[:], op=Alu.min)
                    v = T("v")
                    nc.vector.tensor_tensor(out=v[:], in0=abst[:], in1=sq[:], op=Alu.max)
                    rv = T("rv")
                    nc.vector.reciprocal(rv[:], v[:])
                    rr = T("rr")
                    nc.vector.tensor_tensor(out=rr[:], in0=u[:], in1=rv[:], op=Alu.mult)
                    at = T("at")
                    nc.scalar.activation(out=at[:], in_=rr[:], func=Act.Arctan)
                    pg = T("pg")
                    nc.vector.tensor_scalar(out=pg[:], in0=cos[:], scalar1=0.0,
                                            scalar2=None, op0=Alu.is_gt)
                    ng = T("ng")
                    nc.vector.tensor_scalar(out=ng[:], in0=cos[:], scalar1=0.0,
                                            scalar2=None, op0=Alu.is_lt)
                    sgn = T("sgn")
                    nc.vector.tensor_tensor(out=sgn[:], in0=pg[:], in1=ng[:], op=Alu.subtract)
                    big = T("big")
                    nc.vector.tensor_tensor(out=big[:], in0=abst[:], in1=sq[:], op=Alu.is_gt)
                    c1 = T("c1")
                    nc.vector.tensor_scalar(out=c1[:], in0=big[:], scalar1=2.0,
                                            scalar2=-1.0, op0=Alu.mult, op1=Alu.add)
                    nc.vector.tensor_tensor(out=c1[:], in0=c1[:], in1=sgn[:], op=Alu.mult)
                    c0 = T("c0")
                    nc.vector.tensor_tensor(out=c0[:], in0=big[:], in1=ng[:], op=Alu.mult)
                    nc.vector.tensor_scalar(out=c0[:], in0=c0[:], scalar1=float(np.pi),
                                            scalar2=None, op0=Alu.mult)
                    c0b = T("c0b")
                    nc.vector.tensor_scalar(out=c0b[:], in0=big[:], scalar1=-HALF_PI,
                                            scalar2=HALF_PI, op0=Alu.mult, op1=Alu.add)
                    nc.vector.tensor_tensor(out=c0[:], in0=c0[:], in1=c0b[:], op=Alu.add)
                    ang = T("ang")
                    nc.vector.tensor_tensor(out=ang[:], in0=c1[:], in1=at[:], op=Alu.mult)
                    nc.vector.tensor_tensor(out=ang[:], in0=ang[:], in1=c0[:], op=Alu.add)
                    if stage == 4:
                        nc.vector.tensor_tensor(out=ang[:], in0=ang[:], in1=mask_t, op=Alu.mult)
                        nc.vector.tensor_reduce(
                            out=out_t[:, out_col:out_col + 1], in_=ang[:],
                            axis=mybir.AxisListType.X, op=Alu.add)
                        return
                    # aperture = asin(0.1 / (sqrt(pp) + eps)), small-angle series
                    sp0 = T("sp0")
                    nc.scalar.activation(out=sp0[:], in_=pp_b, func=Act.Sqrt)
                    rp = T("rp")
                    nc.vector.reciprocal(rp[:], sp0[:])
                    sp1 = T("sp1")
                    nc.vector.tensor_tensor(out=sp1[:], in0=pp_b, in1=rp[:], op=Alu.mult)
                    nc.vector.tensor_tensor(out=sp1[:], in0=sp1[:], in1=sp0[:], op=Alu.add)
                    nc.vector.tensor_scalar(out=sp1[:], in0=sp1[:], scalar1=0.5,
                                            scalar2=float(EPS), op0=Alu.mult, op1=Alu.add)
                    rsp = T("rsp")
                    nc.vector.reciprocal(rsp[:], sp1[:])
                    y = T("y")
                    nc.vector.tensor_scalar(out=y[:], in0=rsp[:], scalar1=float(BETA),
                                            scalar2=0.0, op0=Alu.mult, op1=Alu.max)
                    nc.vector.tensor_scalar(out=y[:], in0=y[:], scalar1=float(1.0 - 1e-6),
                                            scalar2=None, op0=Alu.min)
                    y2 = T("y2")
                    nc.vector.tensor_tensor(out=y2[:], in0=y[:], in1=y[:], op=Alu.mult)
                    y3 = T("y3")
                    nc.vector.tensor_tensor(out=y3[:], in0=y2[:], in1=y[:], op=Alu.mult)
                    ap = T("ap")
                    nc.vector.tensor_scalar(out=ap[:], in0=y3[:], scalar1=float(1.0 / 6.0),
                                            scalar2=None, op0=Alu.mult)
                    nc.vector.tensor_tensor(out=ap[:], in0=ap[:], in1=y[:], op=Alu.add)
                    e = T("e")
                    nc.vector.tensor_tensor(out=e[:], in0=ang[:], in1=ap[:], op=Alu.subtract)
                    nc.vector.tensor_scalar(out=e[:], in0=e[:], scalar1=0.0,
                                            scalar2=None, op0=Alu.max)
                    if is_neg:
                        nc.vector.tensor_scalar(out=e[:], in0=e[:], scalar1=-1.0,
                                                scalar2=float(MARGIN), op0=Alu.mult,
                                                op1=Alu.add)
                        nc.vector.tensor_scalar(out=e[:], in0=e[:], scalar1=0.0,
                                                scalar2=None, op0=Alu.max)
                    nc.vector.tensor_tensor(out=e[:], in0=e[:], in1=mask_t, op=Alu.mult)
                    nc.vector.tensor_reduce(
                        out=out_t[:, out_col:out_col + 1], in_=e[:],
                        axis=mybir.AxisListType.X, op=Alu.add)

                grp_p = psgroup("gp", NPB)
                grp_n0 = psgroup("gn0", NH0)
                grp_n1 = psgroup("gn1", NH1)
                stream(posa_t, posb_t, caps_p, cum_p, grp_p, 0, "p",
                       range(NBUCK))
                group_epilogue(grp_p, sb_p, maskp_t[:], NPB, False, 0)
                stream(nega_t, negc_t, caps_n, cum_n, grp_n0, 0, "n",
                       range(NBUCK // 2))
                group_epilogue(grp_n0, sb_n0, maskn_t[:, 0:NH0], NH0, True, 1)
                stream(nega_t, negc_t, caps_n, cum_n, grp_n1, NH0,
                       "n", range(NBUCK // 2, NBUCK))
                group_epilogue(grp_n1, sb_n1, maskn_t[:, NH0:NNB], NH1, True, 2)
                if stage < 3:
                    nc.vector.tensor_reduce(out=out_t[:, 0:1], in_=maskp_t[:],
                                            axis=mybir.AxisListType.X, op=Alu.add)
                    nc.vector.tensor_reduce(out=out_t[:, 1:2], in_=maskn_t[:],
                                            axis=mybir.AxisListType.X, op=Alu.add)
                    nc.vector.tensor_reduce(out=out_t[:, 2:3], in_=maskn_t[:],
                                            axis=mybir.AxisListType.X, op=Alu.add)
                    nc.vector.tensor_reduce(out=out_t[:, 3:4], in_=maskp_t[:],
                                            axis=mybir.AxisListType.X, op=Alu.add)
                nc.sync.dma_start(out=partials[:], in_=out_t[:])

            if loop_iters > 1:
                with tc.For_i(0, loop_iters, 1):
                    loop_body()
            else:
                loop_body()

    nc.compile()
    return nc


def _wrap_idx(q):
    """[n] int16 -> [128, n//16] wrapped+replicated gather-index layout."""
    w = q.reshape(-1, 16).T
    return np.tile(w, (8, 1))


def _prep_stream(a_vals, c_vals, caps):
    """Bucket (a, c) pairs by (a%4, c%4); bucket xy padded to caps[xy] cols.

    Returns int16 quotient idx tiles [128, sum(caps)//16] per role and the
    validity mask [128, sum(caps)//128] in the distributed (partition=col%128,
    block=col//128) layout."""
    cum = np.concatenate([[0], np.cumsum(caps)]).astype(int)
    total = int(cum[-1])
    key = (a_vals % 4) * 4 + (c_vals % 4)
    order = np.argsort(key, kind="stable")
    counts = np.bincount(key, minlength=NBUCK)
    a_q = np.zeros(total, np.int16)
    c_q = np.zeros(total, np.int16)
    mask = np.zeros(total, np.float32)
    off_src = 0
    for xy in range(NBUCK):
        cnt = int(counts[xy])
        assert cnt <= caps[xy], (cnt, caps[xy])
        seg = order[off_src:off_src + cnt]
        off_src += cnt
        off = int(cum[xy])
        a_q[off:off + cnt] = (a_vals[seg] // 4).astype(np.int16)
        c_q[off:off + cnt] = (c_vals[seg] // 4).astype(np.int16)
        mask[off:off + cnt] = 1.0
    a_w = np.concatenate([_wrap_idx(a_q[cum[xy]:cum[xy + 1]])
                          for xy in range(NBUCK)], axis=1)
    c_w = np.concatenate([_wrap_idx(c_q[cum[xy]:cum[xy + 1]])
                          for xy in range(NBUCK)], axis=1)
    nblk = total // 128
    mask_t = mask.reshape(nblk, 128).T.copy()
    return a_w, c_w, mask_t


def _round_cap(x):
    return max(128, ((int(x) + 127) // 128) * 128)


def _prepare(prototypes, pairs, neg_c):
    import ml_dtypes

    prototypes = np.ascontiguousarray(prototypes, dtype=np.float32)
    tblbf = prototypes.astype(ml_dtypes.float8_e4m3 if TBL_FP8 else ml_dtypes.bfloat16)
    pairs = np.asarray(pairs, dtype=np.int32)
    neg_c = np.asarray(neg_c, dtype=np.int32)

    shards = []
    maxp = np.zeros(NBUCK, int)
    maxn = np.zeros(NBUCK, int)
    for k in range(NCORES):
        pk = pairs[k * PPC:(k + 1) * PPC]
        nk = neg_c[k * NPC:(k + 1) * NPC]
        a, b = pk[:, 0], pk[:, 1]
        na = np.repeat(a, K)
        kp = (a % 4) * 4 + (b % 4)
        kn = (na % 4) * 4 + (nk % 4)
        maxp = np.maximum(maxp, np.bincount(kp, minlength=NBUCK))
        maxn = np.maximum(maxn, np.bincount(kn, minlength=NBUCK))
        shards.append((a, b, na, nk))
    caps_p = tuple(_round_cap(x) for x in maxp)
    caps_n = tuple(_round_cap(x) for x in maxn)

    in_maps = []
    for k in range(NCORES):
        a, b, na, nk = shards[k]
        pa, pb, mp = _prep_stream(a, b, caps_p)
        ng_a, ng_c, mn = _prep_stream(na, nk, caps_n)
        in_maps.append({
            "tblbf": tblbf,
            "posa_i": pa, "posb_i": pb,
            "nega_i": ng_a, "negc_i": ng_c,
            "maskp": mp, "maskn": mn,
        })
    return caps_p, caps_n, in_maps


def kernel(prototypes, pairs, neg_c):
    from concourse.bass_utils import run_bass_kernel_spmd

    caps_p, caps_n, in_maps = _prepare(prototypes, pairs, neg_c)
    key = (caps_p, caps_n)
    if key not in _CACHE:
        _CACHE[key] = _build_program(caps_p, caps_n)
    nc = _CACHE[key]

    res = run_bass_kernel_spmd(nc, in_maps, core_ids=list(range(NCORES)))
    pos_sum = 0.0
    neg_sum = 0.0
    for k in range(NCORES):
        part = res.results[k]["partials"]
        pos_sum += float(part[:, 0].sum(dtype=np.float64))
        neg_sum += float(part[:, 1].sum(dtype=np.float64))
        neg_sum += float(part[:, 2].sum(dtype=np.float64))
    loss = 0.5 * (pos_sum / P_TOT + neg_sum / (P_TOT * K))
    return np.float32(loss)



# revision 10
# speedup vs baseline: 1.3259x; 1.3259x over previous
"""EntailmentConeLoss on 8 Trainium2 NeuronCores (v2).

Data-parallel over pairs (8192 pos + 32768 neg per core), prototype table
replicated in bf16 (dots of 256-dim bf16 rows carry ~0.3% error; the loss
averages 327680 energies, tolerance 2e-2).

Per core:
- Rows fetched with gpsimd dma_gather(transpose=True) on a 4-row-strided
  bf16 table view (int16 quotient indices). Pairs bucket-sorted by
  (a%4, c%4), a-residue-major, so ONE A-side gather call covers a whole
  a-residue group (4 buckets for pos, 2 for neg to bound tile size);
  C-side gathers are per bucket. 44 calls/core total (vs 64 naive) cuts
  SWDGE fixed overhead; calls round-robin 4 queues (each queue drives its
  own slice of the 16 DMA engines, so all 4 are needed).
- Elementwise a*c (DVE), a^2 (ACT Square), c^2 (alternating DVE/ACT to
  balance engines) in bf16 -- DVE runs 2-byte tensor_tensor at 2x.
- Reduction over D on the TensorEngine: per 128-pair block the elementwise
  tile is the stationary input and a ones column is moving; psum col =
  global block index, one psum tile per quantity (pp/cc/pc), accumulated
  over the two d-halves. Pair j lands at psum partition j%128, col j//128.
- ONE fused f32 epilogue over the unified [128, NPB+NNB] column space:
  cos = (pc-pp)*Rsqrt(pp*dd), octant-reduced arctan for arccos, arcsin
  series for the aperture, then a hinge relu(mrg + sgn*e) with
  host-precomputed per-column sign/margin/weight tiles that fold the
  pos/neg variants, validity masks and both means into one weighted
  reduce -> partials [128, 1]; host sums across cores.
"""
import os
os.environ.setdefault("NEURON_RT_RESET_CORES", "1")

import numpy as np

C, D = 100000, 256
P_TOT, K = 65536, 4
NCORES = 8
PPC = P_TOT // NCORES          # pos pairs per core
NPC = PPC * K                  # neg pairs per core
NBUCK = 16
EPS = np.float32(1e-6)
BETA = np.float32(0.1)
MARGIN = np.float32(0.1)
QUEUES = 4
GAT_BUFS = int(os.environ.get("KGB", "6"))
EW_BUFS = int(os.environ.get("KEB", "3"))
SQC_MODE = int(os.environ.get("KSQC", "3"))   # sqc on ACT every Nth (0=always ACT)

_CACHE = {}


def _build_program(caps_p, caps_n, loop_iters=1, stage=5):
    import concourse.bass as bass
    import concourse.bacc as bacc
    import concourse.mybir as mybir
    import concourse.tile as tile

    f32 = mybir.dt.float32
    bf16 = mybir.dt.bfloat16
    i16 = mybir.dt.int16
    Alu = mybir.AluOpType
    Act = mybir.ActivationFunctionType

    caps_p = list(caps_p)
    caps_n = list(caps_n)
    cum_p = np.concatenate([[0], np.cumsum(caps_p)]).astype(int)
    cum_n = np.concatenate([[0], np.cumsum(caps_n)]).astype(int)
    NPOS = int(cum_p[-1])
    NNEG = int(cum_n[-1])
    NPB = NPOS // 128
    NNB = NNEG // 128
    NCB = NPB + NNB

    HALF_PI = float(np.float32(np.pi / 2))
    PI = float(np.float32(np.pi))

    nc = bacc.Bacc("TRN2", target_bir_lowering=False, num_devices=NCORES,
                   num_swdge_queues=QUEUES)
    tbl = nc.dram_tensor("tblbf", [C, D], bf16, kind="ExternalInput")
    posa_i = nc.dram_tensor("posa_i", [128, NPOS // 16], i16, kind="ExternalInput")
    posb_i = nc.dram_tensor("posb_i", [128, NPOS // 16], i16, kind="ExternalInput")
    nega_i = nc.dram_tensor("nega_i", [128, NNEG // 16], i16, kind="ExternalInput")
    negc_i = nc.dram_tensor("negc_i", [128, NNEG // 16], i16, kind="ExternalInput")
    wgt_d = nc.dram_tensor("wgt", [128, NCB], f32, kind="ExternalInput")
    sgn_d = nc.dram_tensor("sgn", [128, NCB], f32, kind="ExternalInput")
    mrg_d = nc.dram_tensor("mrg", [128, NCB], f32, kind="ExternalInput")
    partials = nc.dram_tensor("partials", [128, 1], f32, kind="ExternalOutput")

    # A-call segments: pos one per a-residue (4 buckets), neg per a-residue
    # half (2 buckets) to bound SBUF tile size.
    pos_aseg = [(ra, 4 * ra, 4 * ra + 4) for ra in range(4)]
    neg_aseg = [(ra, 4 * ra + h, 4 * ra + h + 2)
                for ra in range(4) for h in (0, 2)]

    with tile.TileContext(nc) as tc:
        with tc.tile_pool(name="io", bufs=1) as io, \
             tc.tile_pool(name="gata", bufs=3) as gata, \
             tc.tile_pool(name="gatc", bufs=GAT_BUFS) as gatc, \
             tc.tile_pool(name="ew", bufs=EW_BUFS) as ew, \
             tc.tile_pool(name="ps", bufs=1, space="PSUM") as ps, \
             tc.tile_pool(name="tmp", bufs=1) as tmp:

            posa_t = io.tile([128, NPOS // 16], i16)
            posb_t = io.tile([128, NPOS // 16], i16)
            nega_t = io.tile([128, NNEG // 16], i16)
            negc_t = io.tile([128, NNEG // 16], i16)
            wgt_t = io.tile([128, NCB], f32)
            sgn_t = io.tile([128, NCB], f32)
            mrg_t = io.tile([128, NCB], f32)
            nc.sync.dma_start(out=posa_t[:], in_=posa_i[:])
            nc.sync.dma_start(out=posb_t[:], in_=posb_i[:])
            nc.sync.dma_start(out=nega_t[:], in_=nega_i[:])
            nc.sync.dma_start(out=negc_t[:], in_=negc_i[:])
            nc.sync.dma_start(out=wgt_t[:], in_=wgt_d[:])
            nc.sync.dma_start(out=sgn_t[:], in_=sgn_d[:])
            nc.sync.dma_start(out=mrg_t[:], in_=mrg_d[:])

            ones_t = io.tile([128, 1], bf16)
            nc.vector.memset(ones_t[:], 1.0)
            out_t = io.tile([128, 1], f32)
            nc.vector.memset(out_t[:], 0.0)

            tview = tbl[:].rearrange("(q r) d -> q r d", r=4)

            qrr = [0]
            sqc_rr = [0]

            def loop_body(_i=None):
                pp_ps = ps.tile([128, NCB], f32, tag="pp", name="pp")
                cc_ps = ps.tile([128, NCB], f32, tag="cc", name="cc")
                pc_ps = ps.tile([128, NCB], f32, tag="pc", name="pc")

                def stream(a_idx_t, c_idx_t, caps, cum, blkbase, asegs):
                    for ra, b0, b1 in asegs:
                        seg = int(cum[b1] - cum[b0])
                        soff16 = int(cum[b0]) // 16
                        A = gata.tile([128, 2, seg], bf16, tag="ga", name="ga")
                        nc.gpsimd.dma_gather(
                            A[:], tview[:, ra, :],
                            a_idx_t[:, soff16:soff16 + seg // 16],
                            seg, seg, D, elem_step=4 * D, transpose=True,
                            single_packet=False, queue_num=qrr[0] % QUEUES)
                        qrr[0] += 1
                        for xy in range(b0, b1):
                            cap = int(caps[xy])
                            off16 = int(cum[xy]) // 16
                            aoff = int(cum[xy] - cum[b0])
                            rc = xy % 4
                            Cc = gatc.tile([128, 2, cap], bf16, tag="gc",
                                           name="gc")
                            nc.gpsimd.dma_gather(
                                Cc[:], tview[:, rc, :],
                                c_idx_t[:, off16:off16 + cap // 16],
                                cap, cap, D, elem_step=4 * D, transpose=True,
                                single_packet=False, queue_num=qrr[0] % QUEUES)
                            qrr[0] += 1
                            if stage < 1:
                                continue
                            Asl = A[:, :, aoff:aoff + cap]
                            prod = ew.tile([128, 2, cap], bf16, tag="pr",
                                           name="pr")
                            nc.vector.tensor_tensor(
                                out=prod[:], in0=Asl, in1=Cc[:], op=Alu.mult)
                            # squares in place (prod already consumed A, C)
                            nc.scalar.activation(
                                out=Asl, in_=Asl, func=Act.Square)
                            sqc_rr[0] += 1
                            if SQC_MODE == 0 or sqc_rr[0] % SQC_MODE == 0:
                                nc.scalar.activation(
                                    out=Cc[:], in_=Cc[:], func=Act.Square)
                            else:
                                nc.vector.tensor_tensor(
                                    out=Cc[:], in0=Cc[:], in1=Cc[:], op=Alu.mult)
                            if stage < 2:
                                continue
                            for t in range(cap // 128):
                                col = blkbase + int(cum[xy]) // 128 + t
                                a0 = aoff + t * 128
                                c0 = t * 128
                                for h, st in ((0, True), (1, False)):
                                    for grp, tl in (
                                            (pp_ps, A[:, h, a0:a0 + 128]),
                                            (cc_ps, Cc[:, h, c0:c0 + 128]),
                                            (pc_ps, prod[:, h, c0:c0 + 128])):
                                        nc.tensor.matmul(
                                            grp[:, col:col + 1],
                                            tl, ones_t[:, 0:1],
                                            start=st, stop=not st)

                stream(posa_t, posb_t, caps_p, cum_p, 0, pos_aseg)
                stream(nega_t, negc_t, caps_n, cum_n, NPB, neg_aseg)

                if stage < 2:
                    nc.vector.tensor_reduce(out=out_t[:], in_=wgt_t[:],
                                            axis=mybir.AxisListType.X, op=Alu.add)
                    nc.sync.dma_start(out=partials[:], in_=out_t[:])
                    return

                # ---------------- unified epilogue ----------------
                T = lambda nm: tmp.tile([128, NCB], f32, tag="ep" + nm,
                                        name="ep" + nm)
                pp = T("pp")
                cc = T("cc")
                pc = T("pc")
                nc.vector.tensor_copy(pp[:], pp_ps[:])
                nc.vector.tensor_copy(cc[:], cc_ps[:])
                nc.vector.tensor_copy(pc[:], pc_ps[:])
                if stage < 3:
                    h = T("h")
                    nc.vector.tensor_tensor(out=h[:], in0=pp[:], in1=wgt_t[:],
                                            op=Alu.mult)
                    nc.vector.tensor_reduce(out=out_t[:], in_=h[:],
                                            axis=mybir.AxisListType.X, op=Alu.add)
                    nc.sync.dma_start(out=partials[:], in_=out_t[:])
                    return

                ppcc = T("ppcc")
                nc.vector.tensor_tensor(out=ppcc[:], in0=pp[:], in1=cc[:], op=Alu.add)
                dd = T("dd")
                nc.vector.scalar_tensor_tensor(
                    out=dd[:], in0=pc[:], scalar=-2.0, in1=ppcc[:],
                    op0=Alu.mult, op1=Alu.add)
                # dup guard: dd is rounding junk when c==p; 1.0 when valid
                dupf = T("dupf")
                nc.vector.scalar_tensor_tensor(
                    out=dupf[:], in0=ppcc[:], scalar=2e-3, in1=dd[:],
                    op0=Alu.mult, op1=Alu.is_lt)
                g = T("g")
                nc.vector.tensor_tensor(out=g[:], in0=pp[:], in1=dd[:], op=Alu.mult)
                nc.vector.tensor_scalar(out=g[:], in0=g[:], scalar1=0.0,
                                        scalar2=1e-30, op0=Alu.max, op1=Alu.add)
                # s1 = sqrt(g) + g/sqrt(g) = 2*sqrt(g) (2nd-order accurate),
                # which is exactly the reference denominator 2*|p|*|diff|.
                s0 = T("s0")
                nc.scalar.activation(out=s0[:], in_=g[:], func=Act.Sqrt)
                r = T("r")
                nc.vector.reciprocal(r[:], s0[:])
                s1 = T("s1")
                nc.vector.tensor_tensor(out=s1[:], in0=g[:], in1=r[:], op=Alu.mult)
                nc.vector.tensor_tensor(out=s1[:], in0=s1[:], in1=s0[:], op=Alu.add)
                rden = T("rden")
                nc.vector.reciprocal(rden[:], s1[:])
                cos = T("cos")
                nc.vector.scalar_tensor_tensor(
                    out=cos[:], in0=pp[:], scalar=-1.0, in1=pc[:],
                    op0=Alu.mult, op1=Alu.add)          # pc - pp
                nc.vector.tensor_tensor(out=cos[:], in0=cos[:], in1=rden[:],
                                        op=Alu.mult)
                # num is really 2*(pc-pp): fold the 2 into the clamp
                nc.vector.tensor_scalar(out=cos[:], in0=cos[:], scalar1=2.0,
                                        scalar2=float(-(1.0 - 1e-6)),
                                        op0=Alu.mult, op1=Alu.max)
                nc.vector.tensor_scalar(out=cos[:], in0=cos[:],
                                        scalar1=float(1.0 - 1e-6),
                                        scalar2=None, op0=Alu.min)
                nc.vector.tensor_tensor(out=cos[:], in0=cos[:], in1=dupf[:], op=Alu.mult)
                # ang = arccos(cos) via octant-reduced arctan
                q = T("q")
                nc.vector.tensor_tensor(out=q[:], in0=cos[:], in1=cos[:], op=Alu.mult)
                nc.vector.tensor_scalar(out=q[:], in0=q[:], scalar1=-1.0,
                                        scalar2=1.0, op0=Alu.mult, op1=Alu.add)
                sq = T("sq")
                nc.scalar.activation(out=sq[:], in_=q[:], func=Act.Sqrt)
                abst = T("abst")
                nc.vector.tensor_scalar(out=abst[:], in0=cos[:], scalar1=-1.0,
                                        scalar2=None, op0=Alu.mult)
                nc.vector.tensor_tensor(out=abst[:], in0=abst[:], in1=cos[:],
                                        op=Alu.max)
                u2 = T("u2")
                nc.vector.tensor_tensor(out=u2[:], in0=abst[:], in1=sq[:], op=Alu.min)
                v = T("v")
                nc.vector.tensor_tensor(out=v[:], in0=abst[:], in1=sq[:], op=Alu.max)
                rv = T("rv")
                nc.vector.reciprocal(rv[:], v[:])
                rr = T("rr")
                nc.vector.tensor_tensor(out=rr[:], in0=u2[:], in1=rv[:], op=Alu.mult)
                at = T("at")
                nc.scalar.activation(out=at[:], in_=rr[:], func=Act.Arctan)
                sgnc = T("sgnc")
                nc.vector.tensor_scalar(out=sgnc[:], in0=cos[:], scalar1=0.0,
                                        scalar2=None, op0=Alu.is_gt)
                ngt = T("ngt")
                nc.vector.tensor_scalar(out=ngt[:], in0=cos[:], scalar1=0.0,
                                        scalar2=None, op0=Alu.is_lt)
                nc.vector.tensor_tensor(out=sgnc[:], in0=sgnc[:], in1=ngt[:],
                                        op=Alu.subtract)
                big = T("big")
                nc.vector.tensor_tensor(out=big[:], in0=abst[:], in1=sq[:],
                                        op=Alu.is_gt)
                c1 = T("c1")
                nc.vector.tensor_scalar(out=c1[:], in0=big[:], scalar1=2.0,
                                        scalar2=-1.0, op0=Alu.mult, op1=Alu.add)
                nc.vector.tensor_tensor(out=c1[:], in0=c1[:], in1=sgnc[:], op=Alu.mult)
                w = T("w")
                nc.vector.tensor_scalar(out=w[:], in0=ngt[:], scalar1=PI,
                                        scalar2=-HALF_PI, op0=Alu.mult, op1=Alu.add)
                c0 = T("c0")
                nc.vector.scalar_tensor_tensor(
                    out=c0[:], in0=big[:], scalar=1.0, in1=w[:],
                    op0=Alu.mult, op1=Alu.mult)
                nc.vector.tensor_scalar(out=c0[:], in0=c0[:], scalar1=1.0,
                                        scalar2=HALF_PI, op0=Alu.mult, op1=Alu.add)
                ang = T("ang")
                nc.vector.tensor_tensor(out=ang[:], in0=c1[:], in1=at[:], op=Alu.mult)
                nc.vector.tensor_tensor(out=ang[:], in0=ang[:], in1=c0[:], op=Alu.add)
                if stage < 5:
                    nc.vector.tensor_tensor(out=ang[:], in0=ang[:], in1=wgt_t[:],
                                            op=Alu.mult)
                    nc.vector.tensor_reduce(out=out_t[:], in_=ang[:],
                                            axis=mybir.AxisListType.X, op=Alu.add)
                    nc.sync.dma_start(out=partials[:], in_=out_t[:])
                    return
                # aperture = asin(beta/|p|), small-angle series. ap ~ 0.006 so
                # raw table sqrt accuracy is plenty (no Newton refinement).
                sp = T("sp")
                nc.scalar.activation(out=sp[:], in_=pp[:], func=Act.Sqrt)
                y = T("y")
                nc.vector.reciprocal(y[:], sp[:])
                nc.vector.tensor_scalar(out=y[:], in0=y[:], scalar1=float(BETA),
                                        scalar2=float(1.0 - 1e-6),
                                        op0=Alu.mult, op1=Alu.min)
                y2 = T("y2")
                nc.vector.tensor_tensor(out=y2[:], in0=y[:], in1=y[:], op=Alu.mult)
                y3 = T("y3")
                nc.vector.tensor_tensor(out=y3[:], in0=y2[:], in1=y[:], op=Alu.mult)
                ap = T("ap")
                nc.vector.scalar_tensor_tensor(
                    out=ap[:], in0=y3[:], scalar=float(1.0 / 6.0), in1=y[:],
                    op0=Alu.mult, op1=Alu.add)
                e = T("e")
                nc.vector.scalar_tensor_tensor(
                    out=e[:], in0=ap[:], scalar=-1.0, in1=ang[:],
                    op0=Alu.mult, op1=Alu.add)
                nc.vector.tensor_scalar(out=e[:], in0=e[:], scalar1=0.0,
                                        scalar2=None, op0=Alu.max)
                # hinge: h = relu(mrg + sgn*e); pos (0,+1)->e, neg (0.1,-1)
                h = T("h")
                nc.vector.tensor_tensor(out=h[:], in0=e[:], in1=sgn_t[:], op=Alu.mult)
                nc.vector.tensor_tensor(out=h[:], in0=h[:], in1=mrg_t[:], op=Alu.add)
                nc.vector.tensor_scalar(out=h[:], in0=h[:], scalar1=0.0,
                                        scalar2=None, op0=Alu.max)
                nc.vector.tensor_tensor(out=h[:], in0=h[:], in1=wgt_t[:], op=Alu.mult)
                nc.vector.tensor_reduce(out=out_t[:], in_=h[:],
                                        axis=mybir.AxisListType.X, op=Alu.add)
                nc.sync.dma_start(out=partials[:], in_=out_t[:])

            if loop_iters > 1:
                with tc.For_i(0, loop_iters, 1):
                    loop_body()
            else:
                loop_body()

    nc.compile()
    return nc


def _wrap_idx(q):
    """[n] int16 -> [128, n//16] wrapped+replicated gather-index layout."""
    w = q.reshape(-1, 16).T
    return np.tile(w, (8, 1))


def _prep_stream(a_vals, c_vals, caps):
    """Bucket (a, c) pairs by (a%4, c%4); bucket xy padded to caps[xy] cols.

    Returns int16 quotient idx tiles [128, sum(caps)//16] per role and the
    validity mask [128, sum(caps)//128] in the distributed (partition=col%128,
    block=col//128) layout. A-indices are wrapped per a-residue segment (the
    merged A-call granularity); C-indices per bucket."""
    cum = np.concatenate([[0], np.cumsum(caps)]).astype(int)
    total = int(cum[-1])
    key = (a_vals % 4) * 4 + (c_vals % 4)
    order = np.argsort(key, kind="stable")
    counts = np.bincount(key, minlength=NBUCK)
    a_q = np.zeros(total, np.int16)
    c_q = np.zeros(total, np.int16)
    mask = np.zeros(total, np.float32)
    off_src = 0
    for xy in range(NBUCK):
        cnt = int(counts[xy])
        assert cnt <= caps[xy], (cnt, caps[xy])
        seg = order[off_src:off_src + cnt]
        off_src += cnt
        off = int(cum[xy])
        a_q[off:off + cnt] = (a_vals[seg] // 4).astype(np.int16)
        c_q[off:off + cnt] = (c_vals[seg] // 4).astype(np.int16)
        mask[off:off + cnt] = 1.0
    # wrapping granularity must match the gather-call segments
    a_w = np.concatenate([_wrap_idx(a_q[cum[s]:cum[e]])
                          for s, e in ((0, 4), (4, 8), (8, 12), (12, 16))], axis=1)
    c_w = np.concatenate([_wrap_idx(c_q[cum[xy]:cum[xy + 1]])
                          for xy in range(NBUCK)], axis=1)
    nblk = total // 128
    mask_t = mask.reshape(nblk, 128).T.copy()
    return a_w, c_w, mask_t


def _prep_stream_neg(a_vals, c_vals, caps):
    """Same as _prep_stream but A-indices wrapped per 2-bucket segment."""
    a_w, c_w, mask_t = _prep_stream(a_vals, c_vals, caps)
    # redo A wrapping at 2-bucket granularity
    cum = np.concatenate([[0], np.cumsum(caps)]).astype(int)
    total = int(cum[-1])
    key = (a_vals % 4) * 4 + (c_vals % 4)
    order = np.argsort(key, kind="stable")
    counts = np.bincount(key, minlength=NBUCK)
    a_q = np.zeros(total, np.int16)
    off_src = 0
    for xy in range(NBUCK):
        cnt = int(counts[xy])
        seg = order[off_src:off_src + cnt]
        off_src += cnt
        a_q[int(cum[xy]):int(cum[xy]) + cnt] = (a_vals[seg] // 4).astype(np.int16)
    a_w = np.concatenate([_wrap_idx(a_q[cum[s]:cum[s + 2]])
                          for s in range(0, NBUCK, 2)], axis=1)
    return a_w, c_w, mask_t


def _round_cap(x):
    return max(128, ((int(x) + 127) // 128) * 128)


def _prepare(prototypes, pairs, neg_c):
    import ml_dtypes

    prototypes = np.ascontiguousarray(prototypes, dtype=np.float32)
    tblbf = prototypes.astype(ml_dtypes.bfloat16)
    pairs = np.asarray(pairs, dtype=np.int32)
    neg_c = np.asarray(neg_c, dtype=np.int32)

    shards = []
    maxp = np.zeros(NBUCK, int)
    maxn = np.zeros(NBUCK, int)
    for k in range(NCORES):
        pk = pairs[k * PPC:(k + 1) * PPC]
        nk = neg_c[k * NPC:(k + 1) * NPC]
        a, b = pk[:, 0], pk[:, 1]
        na = np.repeat(a, K)
        kp = (a % 4) * 4 + (b % 4)
        kn = (na % 4) * 4 + (nk % 4)
        maxp = np.maximum(maxp, np.bincount(kp, minlength=NBUCK))
        maxn = np.maximum(maxn, np.bincount(kn, minlength=NBUCK))
        shards.append((a, b, na, nk))
    caps_p = tuple(_round_cap(x) for x in maxp)
    caps_n = tuple(_round_cap(x) for x in maxn)
    NPB = sum(caps_p) // 128
    NNB = sum(caps_n) // 128

    in_maps = []
    for k in range(NCORES):
        a, b, na, nk = shards[k]
        pa, pb, mp = _prep_stream(a, b, caps_p)
        ng_a, ng_c, mn = _prep_stream_neg(na, nk, caps_n)
        wgt = np.concatenate(
            [mp * np.float32(0.5 / P_TOT), mn * np.float32(0.5 / (P_TOT * K))],
            axis=1)
        sgn = np.concatenate(
            [np.full((128, NPB), 1.0, np.float32),
             np.full((128, NNB), -1.0, np.float32)], axis=1)
        mrg = np.concatenate(
            [np.zeros((128, NPB), np.float32),
             np.full((128, NNB), float(MARGIN), np.float32)], axis=1)
        in_maps.append({
            "tblbf": tblbf,
            "posa_i": pa, "posb_i": pb,
            "nega_i": ng_a, "negc_i": ng_c,
            "wgt": wgt, "sgn": sgn, "mrg": mrg,
        })
    return caps_p, caps_n, in_maps


def kernel(prototypes, pairs, neg_c):
    from concourse.bass_utils import run_bass_kernel_spmd

    caps_p, caps_n, in_maps = _prepare(prototypes, pairs, neg_c)
    key = (caps_p, caps_n)
    if key not in _CACHE:
        _CACHE[key] = _build_program(caps_p, caps_n)
    nc = _CACHE[key]

    res = run_bass_kernel_spmd(nc, in_maps, core_ids=list(range(NCORES)))
    tot = 0.0
    for k in range(NCORES):
        tot += float(res.results[k]["partials"].sum(dtype=np.float64))
    return np.float32(tot)
